# revision 3
# baseline (speedup 1.0000x reference)
"""Trainium2 Bass kernel for ChunkLayerReference (ragged sequence compaction).

Problem: for each batch row, stably compact boundary-selected tokens to the
front; output [B, M, D] where M = max_b(num_selected[b]), plus validity mask.
Semantics match a stable argsort of (i + (1-mask)*L) truncated to M — i.e. the
first M entries of [selected tokens in order, unselected tokens in order].

Sharding: data parallel over B=8 across the 8 NeuronCores (one row per core,
no cross-core communication).

Device algorithm (per core, one batch row of L=8192 tokens, D=1024 f32):
  Phase A (index build, ~15-20us):
    - mask laid out [128, 64] (token i at [i//64, i%64])
    - exclusive flat cumsum via DVE scan (free dim) + strict-lower-triangular
      PE matmul (partition dim)
    - slot[i] = mask ? csum[i] : n_sel + i - csum[i]  (forward permutation)
    - inverse permutation via one-hot factorization: for output j = jhi*128+jlo,
      inv[jlo, jhi] = sum_i [slot_lo[i]==jlo] * (i * [slot_hi[i]==jhi]),
      accumulated over 64 i-tiles as PE matmuls into PSUM. Exact in f32.
  Phase B (gather, HBM-bandwidth bound):
    - 128 rows per indirect DMA gather (HBM -> SBUF) using inv column as
      per-partition row offsets; stores batched 4 blocks (2 MiB) per HWDGE DMA.
"""

import numpy as np

import concourse.bass as bass
import concourse.bacc as bacc
import concourse.mybir as mybir
import concourse.tile as tile
from concourse.bass_utils import run_bass_kernel_spmd

P = 128
B, L, D = 8, 8192, 1024
C = L // P  # 64 free-dim columns for the token layout
GB = 4      # gather blocks (128 rows each) per store DMA

f32 = mybir.dt.float32
i32 = mybir.dt.int32
u8 = mybir.dt.uint8

# test.py reads this for the HW exec time of the last run.
LAST_RESULTS = None


def _build(M: int, reps: int = 1) -> bacc.Bacc:
    NBLK = (M + P - 1) // P     # output blocks of 128 rows
    FBLK = M // P               # full blocks
    RREM = M - FBLK * P         # rows in the final partial block

    nc = bacc.Bacc(
        "TRN2",
        target_bir_lowering=False,
        debug=False,
        enable_asserts=False,
        num_devices=B,
    )
    hid = nc.dram_tensor("hidden", [L, D], f32, kind="ExternalInput").ap()
    msk = nc.dram_tensor("mask", [P, C], u8, kind="ExternalInput").ap()
    out = nc.dram_tensor("out", [M, D], f32, kind="ExternalOutput").ap()

    add = mybir.AluOpType.add
    sub = mybir.AluOpType.subtract
    mult = mybir.AluOpType.mult
    eq_op = mybir.AluOpType.is_equal

    with tile.TileContext(nc) as tc:
        with (
            tc.tile_pool(name="consts", bufs=1) as cpool,
            tc.tile_pool(name="phasea", bufs=1) as apool,
            tc.tile_pool(name="olo", bufs=3) as opool,
            tc.tile_pool(name="eqa", bufs=3) as epool,
            tc.tile_pool(name="psum", bufs=1, space="PSUM") as pspool,
            tc.tile_pool(name="gather", bufs=4) as gpool,
        ):
            for rep in range(reps):
                if rep > 0:
                    tc.strict_bb_all_engine_barrier()
                _body(nc, tc, cpool, apool, opool, epool, pspool, gpool,
                      hid, msk, out, M, NBLK, FBLK, RREM,
                      add, sub, mult, eq_op)

    nc.compile()
    return nc


def _body(nc, tc, cpool, apool, opool, epool, pspool, gpool,
          hid, msk, out, M, NBLK, FBLK, RREM, add, sub, mult, eq_op):
        if True:
            # ---- constants ----
            iota_flat = cpool.tile([P, C], f32)  # i = p*C + c
            nc.gpsimd.iota(iota_flat[:], pattern=[[1, C]], base=0,
                           channel_multiplier=C,
                           allow_small_or_imprecise_dtypes=True)
            iota128 = cpool.tile([P, P], f32)  # 0..127 per row
            nc.gpsimd.iota(iota128[:], pattern=[[1, P]], base=0,
                           channel_multiplier=0,
                           allow_small_or_imprecise_dtypes=True)
            iotaJ = cpool.tile([P, NBLK], f32)  # 0..NBLK-1 per row
            nc.gpsimd.iota(iotaJ[:], pattern=[[1, NBLK]], base=0,
                           channel_multiplier=0,
                           allow_small_or_imprecise_dtypes=True)
            ones = cpool.tile([P, P], f32)
            nc.vector.memset(ones[:], 1.0)
            # strict lower triangular (as lhsT): LT[k, p] = 1 if k < p
            lt = cpool.tile([P, P], f32)
            nc.gpsimd.memset(lt[:], 1.0)
            nc.gpsimd.affine_select(
                out=lt[:], in_=lt[:], pattern=[[1, P]], base=0,
                channel_multiplier=-1, compare_op=mybir.AluOpType.is_gt,
                fill=0.0)
            zeros = cpool.tile([P, C], f32)
            nc.vector.memset(zeros[:], 0.0)

            # ---- phase A: forward permutation slot[i] ----
            m_u8 = apool.tile([P, C], u8)
            nc.sync.dma_start(out=m_u8[:], in_=msk[:])
            m = apool.tile([P, C], f32)
            nc.vector.tensor_copy(m[:], m_u8[:])

            incl = apool.tile([P, C], f32)  # inclusive within-partition cumsum
            nc.vector.tensor_tensor_scan(
                out=incl[:], data0=m[:], data1=zeros[:], initial=0.0,
                op0=add, op1=add)

            s = incl[:, C - 1:C]  # per-partition sums [P,1]
            cs_ps = pspool.tile([P, 1], f32, space="PSUM", tag="cs")
            nc.tensor.matmul(cs_ps[:], lhsT=lt[:], rhs=s, start=True, stop=True)
            tot_ps = pspool.tile([P, 1], f32, space="PSUM", tag="tot")
            nc.tensor.matmul(tot_ps[:], lhsT=ones[:], rhs=s, start=True,
                             stop=True)

            # exclusive flat cumsum: csum = (incl - m) + cs[p]
            csum = apool.tile([P, C], f32)
            nc.vector.tensor_tensor(csum[:], incl[:], m[:], sub)
            nc.vector.tensor_scalar(csum[:], csum[:], cs_ps[:, 0:1], None, add)

            # t = tot + i - csum ; slot = m*csum + (1-m)*t
            t = apool.tile([P, C], f32)
            nc.vector.tensor_tensor(t[:], iota_flat[:], csum[:], sub)
            nc.vector.tensor_scalar(t[:], t[:], tot_ps[:, 0:1], None, add)
            w = apool.tile([P, C], f32)
            nc.vector.tensor_scalar(w[:], m[:], -1.0, 1.0, mult, add)  # 1-m
            slot = apool.tile([P, C], f32)
            nc.vector.tensor_tensor(slot[:], csum[:], m[:], mult)
            nc.vector.tensor_tensor(t[:], t[:], w[:], mult)
            nc.vector.tensor_tensor(slot[:], slot[:], t[:], add)

            # split slot into (hi, lo) base-128 digits
            slot_i = apool.tile([P, C], i32)
            nc.vector.tensor_copy(slot_i[:], slot[:])
            lo_i = apool.tile([P, C], i32)
            nc.vector.tensor_scalar(lo_i[:], slot_i[:], 127, None,
                                    mybir.AluOpType.bitwise_and)
            hi_i = apool.tile([P, C], i32)
            nc.vector.tensor_scalar(hi_i[:], slot_i[:], 7, None,
                                    mybir.AluOpType.arith_shift_right)
            slot_lo = apool.tile([P, C], f32)
            nc.vector.tensor_copy(slot_lo[:], lo_i[:])
            slot_hi = apool.tile([P, C], f32)
            nc.vector.tensor_copy(slot_hi[:], hi_i[:])

            # ---- phase A: inverse permutation via PE ----
            inv_ps = pspool.tile([P, NBLK], f32, space="PSUM", tag="inv")
            for c in range(C):
                o_lo = opool.tile([P, P], f32)
                nc.vector.tensor_tensor(
                    o_lo[:], slot_lo[:, c:c + 1].to_broadcast([P, P]),
                    iota128[:], eq_op)
                a = epool.tile([P, NBLK], f32)
                nc.vector.tensor_tensor(
                    a[:], slot_hi[:, c:c + 1].to_broadcast([P, NBLK]),
                    iotaJ[:], eq_op)
                nc.vector.tensor_scalar(a[:], a[:], iota_flat[:, c:c + 1],
                                        None, mult)
                nc.tensor.matmul(inv_ps[:], lhsT=o_lo[:], rhs=a[:],
                                 start=(c == 0), stop=(c == C - 1))

            inv = apool.tile([P, NBLK], i32)
            nc.vector.tensor_copy(inv[:], inv_ps[:])

            # ---- phase B: gather + store ----
            # out rows viewed [p, blk, d] with row = blk*128 + p
            out_blk = out[:FBLK * P, :].rearrange("(blk p) d -> p blk d", p=P)
            for g in range((NBLK + GB - 1) // GB):
                nb = min(GB, NBLK - g * GB)
                gt = gpool.tile([P, GB * D], f32, tag="gt")
                for k in range(nb):
                    cblk = g * GB + k
                    nc.gpsimd.indirect_dma_start(
                        out=gt[:, k * D:(k + 1) * D],
                        out_offset=None,
                        in_=hid[:],
                        in_offset=bass.IndirectOffsetOnAxis(
                            ap=inv[:, cblk:cblk + 1], axis=0),
                    )
                nfull = min(nb, max(0, FBLK - g * GB))
                if nfull > 0:
                    nc.sync.dma_start(
                        out=out_blk[:, g * GB:g * GB + nfull, :],
                        in_=gt[:, :nfull * D].rearrange(
                            "p (blk d) -> p blk d", d=D),
                    )
                if RREM > 0 and g * GB <= FBLK < g * GB + nb:
                    k = FBLK - g * GB
                    nc.sync.dma_start(
                        out=out[FBLK * P:M, :],
                        in_=gt[:RREM, k * D:(k + 1) * D],
                    )


_CACHE: dict[int, bacc.Bacc] = {}


def kernel(hidden_states: np.ndarray, boundary_mask: np.ndarray):
    global LAST_RESULTS
    hs = np.ascontiguousarray(np.asarray(hidden_states, dtype=np.float32))
    bm = np.asarray(boundary_mask).astype(bool)
    assert hs.shape == (B, L, D) and bm.shape == (B, L)

    num_tokens = bm.sum(axis=-1).astype(np.int64)
    M = int(num_tokens.max())
    next_mask = (np.arange(M)[None, :] < num_tokens[:, None])
    if M == 0:
        return np.zeros((B, 0, D), np.float32), next_mask

    if M not in _CACHE:
        _CACHE[M] = _build(M)
    nc = _CACHE[M]

    in_maps = [
        {
            "hidden": hs[b],
            "mask": np.ascontiguousarray(bm[b].astype(np.uint8).reshape(P, C)),
        }
        for b in range(B)
    ]
    res = run_bass_kernel_spmd(nc, in_maps, core_ids=list(range(B)))
    LAST_RESULTS = res

    next_hidden = np.stack([res.results[b]["out"] for b in range(B)], axis=0)
    return next_hidden, next_mask


# revision 10
# speedup vs baseline: 1.3517x; 1.3517x over previous
"""Trainium2 Bass kernel for ChunkLayerReference (ragged sequence compaction).

Problem: for each batch row, stably compact boundary-selected tokens to the
front; output [B, M, D] where M = max_b(num_selected[b]), plus validity mask.
Semantics match a stable argsort of (i + (1-mask)*L) truncated to M — i.e. the
first M entries of [selected tokens in order, unselected tokens in order].

Sharding: data parallel over B=8 across the 8 NeuronCores (one row per core,
no cross-core communication).

Device algorithm (per core, one batch row of L=8192 tokens, D=1024 f32):
  Phase A (index build):
    - mask laid out [128, 64] (token i at [i//64, i%64])
    - exclusive flat cumsum via DVE scan (free dim) + strict-lower-triangular
      PE matmul (partition dim)
    - slot[i] = mask ? csum[i] : n_sel + i - csum[i]  (forward permutation)
    - inverse permutation via one-hot factorization: for output j = jhi*128+jlo,
      inv[jlo, jhi] = sum_i [slot_lo[i]==jlo] * (i * [slot_hi[i]==jhi]),
      accumulated over 64 i-tiles as PE matmuls into PSUM. Exact in f32.
  Phase B (gather, HBM-bandwidth bound):
    - GB*128 rows per indirect DMA gather (HBM -> SBUF) using inv columns as
      per-partition row offsets; each group stored with one HWDGE DMA.
"""

import numpy as np

import concourse.bass as bass
import concourse.bacc as bacc
import concourse.mybir as mybir
import concourse.tile as tile
from concourse.bass_utils import run_bass_kernel_spmd

P = 128
B, L, D = 8, 8192, 1024
C = L // P  # 64 free-dim columns for the token layout
GB = 4      # gather blocks (128 rows each) per gather/store DMA

f32 = mybir.dt.float32
i32 = mybir.dt.int32
u8 = mybir.dt.uint8

ADD = mybir.AluOpType.add
SUB = mybir.AluOpType.subtract
MULT = mybir.AluOpType.mult
EQ = mybir.AluOpType.is_equal

# test.py reads this for the HW exec time of the last run.
LAST_RESULTS = None


def _make_consts(nc, cpool, NBLK):
    cst = {}
    cst["iota_flat"] = cpool.tile([P, C], f32, tag="iota_flat", name="iota_flat")  # i = p*C + c
    nc.gpsimd.iota(cst["iota_flat"][:], pattern=[[1, C]], base=0,
                   channel_multiplier=C, allow_small_or_imprecise_dtypes=True)
    cst["iota128"] = cpool.tile([P, P], f32, tag="iota128", name="iota128")  # 0..127 per row
    nc.gpsimd.iota(cst["iota128"][:], pattern=[[1, P]], base=0,
                   channel_multiplier=0, allow_small_or_imprecise_dtypes=True)
    cst["iotaJ"] = cpool.tile([P, NBLK], f32, tag="iotaJ", name="iotaJ")  # 0..NBLK-1 per row
    nc.gpsimd.iota(cst["iotaJ"][:], pattern=[[1, NBLK]], base=0,
                   channel_multiplier=0, allow_small_or_imprecise_dtypes=True)
    cst["ones"] = cpool.tile([P, P], f32, tag="ones", name="ones")
    nc.vector.memset(cst["ones"][:], 1.0)
    # strict lower triangular (as lhsT): LT[k, p] = 1 if k < p
    cst["lt"] = cpool.tile([P, P], f32, tag="lt", name="lt")
    nc.gpsimd.memset(cst["lt"][:], 1.0)
    nc.gpsimd.affine_select(
        out=cst["lt"][:], in_=cst["lt"][:], pattern=[[1, P]], base=0,
        channel_multiplier=-1, compare_op=mybir.AluOpType.is_gt, fill=0.0)
    cst["zeros"] = cpool.tile([P, C], f32, tag="zeros", name="zeros")
    nc.vector.memset(cst["zeros"][:], 0.0)
    return cst


def _index_build(nc, cst, apool, opool, epool, pspool, msk, inv, NBLK):
    """Compute inv[jlo, jhi] = input token index for output row jhi*128+jlo."""
    m_u8 = apool.tile([P, C], u8, tag="m_u8")
    nc.sync.dma_start(out=m_u8[:], in_=msk[:])
    m = apool.tile([P, C], f32, tag="m")
    nc.vector.tensor_copy(m[:], m_u8[:])

    incl = apool.tile([P, C], f32, tag="incl")  # within-partition incl. cumsum
    nc.vector.tensor_tensor_scan(
        out=incl[:], data0=m[:], data1=cst["zeros"][:], initial=0.0,
        op0=ADD, op1=ADD)

    s = incl[:, C - 1:C]  # per-partition sums [P,1]
    cs_ps = pspool.tile([P, 1], f32, space="PSUM", tag="cs")
    nc.tensor.matmul(cs_ps[:], lhsT=cst["lt"][:], rhs=s, start=True, stop=True)
    tot_ps = pspool.tile([P, 1], f32, space="PSUM", tag="tot")
    nc.tensor.matmul(tot_ps[:], lhsT=cst["ones"][:], rhs=s, start=True,
                     stop=True)

    # exclusive flat cumsum: csum = (incl - m) + cs[p]
    csum = apool.tile([P, C], f32, tag="csum")
    nc.vector.tensor_tensor(csum[:], incl[:], m[:], SUB)
    nc.vector.tensor_scalar(csum[:], csum[:], cs_ps[:, 0:1], None, ADD)

    # t = tot + i - csum ; slot = m*csum + (1-m)*t
    t = apool.tile([P, C], f32, tag="t")
    nc.vector.tensor_tensor(t[:], cst["iota_flat"][:], csum[:], SUB)
    nc.vector.tensor_scalar(t[:], t[:], tot_ps[:, 0:1], None, ADD)
    w = apool.tile([P, C], f32, tag="w")
    nc.vector.tensor_scalar(w[:], m[:], -1.0, 1.0, MULT, ADD)  # 1-m
    slot = apool.tile([P, C], f32, tag="slot")
    nc.vector.tensor_tensor(slot[:], csum[:], m[:], MULT)
    nc.vector.tensor_tensor(t[:], t[:], w[:], MULT)
    nc.vector.tensor_tensor(slot[:], slot[:], t[:], ADD)

    # split slot into (hi, lo) base-128 digits
    slot_i = apool.tile([P, C], i32, tag="slot_i")
    nc.vector.tensor_copy(slot_i[:], slot[:])
    lo_i = apool.tile([P, C], i32, tag="lo_i")
    nc.vector.tensor_scalar(lo_i[:], slot_i[:], 127, None,
                            mybir.AluOpType.bitwise_and)
    hi_i = apool.tile([P, C], i32, tag="hi_i")
    nc.vector.tensor_scalar(hi_i[:], slot_i[:], 7, None,
                            mybir.AluOpType.arith_shift_right)
    slot_lo = apool.tile([P, C], f32, tag="slot_lo")
    nc.vector.tensor_copy(slot_lo[:], lo_i[:])
    slot_hi = apool.tile([P, C], f32, tag="slot_hi")
    nc.vector.tensor_copy(slot_hi[:], hi_i[:])

    # inverse permutation via PE accumulation over the 64 token tiles
    inv_ps = pspool.tile([P, NBLK], f32, space="PSUM", tag="inv")
    for c in range(C):
        o_lo = opool.tile([P, P], f32, tag="o_lo")
        nc.vector.tensor_tensor(
            o_lo[:], slot_lo[:, c:c + 1].to_broadcast([P, P]),
            cst["iota128"][:], EQ)
        a = epool.tile([P, NBLK], f32, tag="a")
        nc.vector.tensor_tensor(
            a[:], slot_hi[:, c:c + 1].to_broadcast([P, NBLK]),
            cst["iotaJ"][:], EQ)
        nc.vector.tensor_scalar(a[:], a[:], cst["iota_flat"][:, c:c + 1],
                                None, MULT)
        nc.tensor.matmul(inv_ps[:], lhsT=o_lo[:], rhs=a[:],
                         start=(c == 0), stop=(c == C - 1))

    nc.vector.tensor_copy(inv[:], inv_ps[:])


def _gather_store(nc, gpool, hid, out, inv, M, NBLK, FBLK, RREM):
    # out rows viewed [p, blk, d] with row = blk*128 + p
    out_blk = out[:FBLK * P, :].rearrange("(blk p) d -> p blk d", p=P)
    for g in range((NBLK + GB - 1) // GB):
        gs = g * GB
        nb = min(GB, NBLK - gs)
        gt = gpool.tile([P, GB * D], f32, tag="gt")
        # one indirect DMA per 128-row block: offsets [128, 1]; HW pairs
        # offset[p] with the p-th 4KB chunk of the dest AP. (A batched
        # [128, nb] offset AP is consumed partition-fastest by the HW DGE
        # while dest chunks go partition-slowest, so it scrambles rows.)
        for k in range(nb):
            nc.gpsimd.indirect_dma_start(
                out=gt[:, k * D:(k + 1) * D],
                out_offset=None,
                in_=hid[:],
                in_offset=bass.IndirectOffsetOnAxis(
                    ap=inv[:, gs + k:gs + k + 1], axis=0),
            )
        nfull = min(nb, max(0, FBLK - gs))
        if nfull > 0:
            nc.sync.dma_start(
                out=out_blk[:, gs:gs + nfull, :],
                in_=gt[:, :nfull * D].rearrange("p (blk d) -> p blk d", d=D),
            )
        if RREM > 0 and gs <= FBLK < gs + nb:
            k = FBLK - gs
            nc.sync.dma_start(
                out=out[FBLK * P:M, :],
                in_=gt[:RREM, k * D:(k + 1) * D],
            )


def _build(M: int, reps: int = 1, phase: str = "all") -> bacc.Bacc:
    """phase: 'all' | 'a' (index build only) | 'b' (gather/store with dummy
    indices) — the partial variants exist only for micro-benchmarking."""
    NBLK = (M + P - 1) // P     # output blocks of 128 rows
    FBLK = M // P               # full blocks
    RREM = M - FBLK * P         # rows in the final partial block

    nc = bacc.Bacc(
        "TRN2",
        target_bir_lowering=False,
        debug=False,
        enable_asserts=False,
        num_devices=B,
    )
    hid = nc.dram_tensor("hidden", [L, D], f32, kind="ExternalInput").ap()
    msk = nc.dram_tensor("mask", [P, C], u8, kind="ExternalInput").ap()
    out = nc.dram_tensor("out", [M, D], f32, kind="ExternalOutput").ap()

    with tile.TileContext(nc) as tc:
        with (
            tc.tile_pool(name="consts", bufs=1) as cpool,
            tc.tile_pool(name="phasea", bufs=1) as apool,
            tc.tile_pool(name="olo", bufs=3) as opool,
            tc.tile_pool(name="eqa", bufs=3) as epool,
            tc.tile_pool(name="psum", bufs=1, space="PSUM") as pspool,
            tc.tile_pool(name="gather", bufs=4) as gpool,
        ):
            for rep in range(reps):
                if rep > 0:
                    tc.strict_bb_all_engine_barrier()
                cst = _make_consts(nc, cpool, NBLK)
                inv = apool.tile([P, NBLK], i32, tag="inv_i")
                if phase == "b":
                    # dummy indices: inv[p, c] = c*128 + p (identity)
                    nc.gpsimd.iota(inv[:], pattern=[[P, NBLK]], base=0,
                                   channel_multiplier=1,
                                   allow_small_or_imprecise_dtypes=True)
                if phase in ("all", "a"):
                    _index_build(nc, cst, apool, opool, epool, pspool,
                                 msk, inv, NBLK)
                if phase in ("all", "b"):
                    _gather_store(nc, gpool, hid, out, inv, M, NBLK, FBLK,
                                  RREM)

    nc.compile()
    return nc


_CACHE: dict[int, bacc.Bacc] = {}


def kernel(hidden_states: np.ndarray, boundary_mask: np.ndarray):
    global LAST_RESULTS
    hs = np.ascontiguousarray(np.asarray(hidden_states, dtype=np.float32))
    bm = np.asarray(boundary_mask).astype(bool)
    assert hs.shape == (B, L, D) and bm.shape == (B, L)

    num_tokens = bm.sum(axis=-1).astype(np.int64)
    M = int(num_tokens.max())
    next_mask = (np.arange(M)[None, :] < num_tokens[:, None])
    if M == 0:
        return np.zeros((B, 0, D), np.float32), next_mask

    if M not in _CACHE:
        _CACHE[M] = _build(M)
    nc = _CACHE[M]

    in_maps = [
        {
            "hidden": hs[b],
            "mask": np.ascontiguousarray(bm[b].astype(np.uint8).reshape(P, C)),
        }
        for b in range(B)
    ]
    res = run_bass_kernel_spmd(nc, in_maps, core_ids=list(range(B)))
    LAST_RESULTS = res

    next_hidden = np.stack([res.results[b]["out"] for b in range(B)], axis=0)
    return next_hidden, next_mask
